# revision 94
# baseline (speedup 1.0000x reference)
"""Trainium2 Bass kernel for nn_CAModel (sobel-conv + 2-layer MLP + masked
residual).

Math per pixel: y = [x, sobel_x(x), sobel_y(x)] (48 ch); h = relu(w0 @ y + b0);
u = w1 @ h; out = x + u * (rand_u > 0.5).

Sharding: pure data-parallel over 8 cores: (batch b, H-half) -> core b*2 + half.
Each core computes a [16, 256, 512] slice of the output.

Design (v3): this environment is DMA-transfer bound (~120 GB/s effective,
all queues serialized), so everything is batched into few, large DMAs:
- mm1 as ONE K=96 matmul per row: Y96 = [A;B | x;B | A;B] where A = Sy x
  (vertical smooth), B = Dy x (vertical diff), computed by 3 DVE ops.
  Horizontal sobel shifts are baked into the pack DMAs via flat column
  offsets (Y stored flat [96, YR*514+2]; block pair at flat offset 1-shift).
  Block pairs are channel-interleaved (partition = 16*pair + 2c + slab) so
  each pack DMA has a plain partition-range destination.
- Residual x from a host-packed DRAM tensor in the stacked [32k+c] layout,
  loaded 16-partitions-per-block into a persistent double-wide tile whose
  junk partitions are primed once (v4: halves xs DMA bytes).
- Mask: host-thresholded compact fp8 [n_rch, 4, RCH, W] tensor, replicated
  on-chip with stride-0 partition-broadcast DMAs (v4: no device compare,
  no DRAM round-trip, half the mask bytes).
- Output: bf16 [yt, k, 16, tl, w]; only the 16 real channels of each
  32-block are DMA'd out (4 DMAs per y-tile); host converts to f32 and
  reassembles.
- Evac (PSUM->SBUF relu+bias) in FD=1024 ops split ACT/DVE (1/5 DVE);
  mask-multiply on DVE; residual add on GpSimd; out DMAs deferred one
  y-tile to avoid head-of-line blocking on the SP DMA queue.

HW attribution (ablations, per 326us iter): input-side DMAs ~92us,
Y-pack SBUF->SBUF ~189us (25.3MB at ~135GB/s effective), mm1+evac ~66us,
mm2/UM/OFY hidden, out DMAs ~47us. Effective DMA bandwidth is ~135-180
GB/s (not the 360 the cost model assumes), so bytes moved dominate.
Measured dead ends: fp8 Y-pack (gradients carry 96% of h variance ->
3.6% rel err, fails 2e-2), K=64+32 Y dedup (PE stationary-swap drains
cost more than the 8.4MB saved; PSUM zero-region rules forbid batched
accumulation), pre-replicated 8MB host mask (extra HBM reads), coarser
XS/REP chunks (hurt prefetch pipelining), sup_half drain-hiding (no
measurable gain), DVE partition-broadcast mask-on-h (stride-0 partition
APs illegal on compute engines), 2-y-tile out batching (neutral).
Run-to-run HW noise is +/-20us across processes.

Final config (CFG): out_split + xs_slim + fp8 compact host mask +
ybufs=3 (triple-buffered Y-pack). Best measured: 303.7us; typical
~304-325us vs 326-355us for the original baseline re-measured in the
same sessions.
"""
import numpy as np
from contextlib import ExitStack

import concourse.bass as bass
import concourse.bacc as bacc
import concourse.tile as tile
from concourse import mybir

bf16 = mybir.dt.bfloat16
f32 = mybir.dt.float32
Alu = mybir.AluOpType
Act = mybir.ActivationFunctionType

C = 16          # channels
HID = 128
N_CORES = 8
UT = 4          # rows per u-tile


def build_nc(R=256, W=512, GR=32, YR=16, reps=1, ablate=(), cut=0,
             rep_q="gpsimd", out_split=True, xs_slim=False, xs_q="gpsimd",
             mask_fp8=False, ypack_qs=("sync", "scalar", "gpsimd"),
             evac_mod=5, evac_ph=2, xch=8, rch=8, hbufs=4, y_fp8=False,
             y2=False, sup_half=False, out2=False, ybufs=2):
    """Per-core graph. R out rows, W out cols, GR rows/group, YR rows/y-tile."""
    WP = W + 2
    n_grp = R // GR
    n_yt = R // YR
    n_ut = R // UT
    UPY = YR // UT                  # u-tiles per y-tile
    XCH = min(xch, n_ut)            # u-tiles per XS chunk
    n_xch = n_ut // XCH
    RCH = min(rch, n_ut)            # u-tiles per REP tile
    n_rch = n_ut // RCH
    SLAB = (GR + 2) * WP            # supertile slab elems per partition
    assert n_grp * C <= 128 and YR <= GR and GR % YR == 0 and YR % UT == 0

    nc = bacc.Bacc()
    XC = 16 if xs_slim else 32
    mdt = mybir.dt.float8e4 if mask_fp8 else bf16
    ydt = mybir.dt.float8e4 if y_fp8 else bf16
    x_ext = nc.declare_dram_parameter("x", (n_grp * C, GR + 2, WP), ydt,
                                      isOutput=False)
    xs_ext = nc.declare_dram_parameter("xs", (n_xch, 4, XC, XCH, W), bf16,
                                       isOutput=False)
    mrep_ext = nc.declare_dram_parameter("mrep", (n_rch, 4, RCH, W), mdt,
                                         isOutput=False)
    wabc_ext = nc.declare_dram_parameter("wabc", (96, HID), ydt,
                                         isOutput=False)
    w1x4_ext = nc.declare_dram_parameter("w1x4", (HID, 128), bf16,
                                         isOutput=False)
    b0_ext = nc.declare_dram_parameter("b0", (HID, 1), f32, isOutput=False)
    OC = 16 if out_split else 32
    if out2:
        assert out_split and n_yt % 2 == 0
        out_ext = nc.declare_dram_parameter(
            "out", (n_yt // 2, 4, 16, 2, UPY, W), bf16, isOutput=True)
    else:
        out_ext = nc.declare_dram_parameter(
            "out", (n_yt, 4, OC, UPY, W), bf16, isOutput=True)

    with tile.TileContext(nc) as tc, ExitStack() as ctx:
        const = ctx.enter_context(tc.tile_pool(name="const", bufs=1))
        big = ctx.enter_context(tc.tile_pool(name="big", bufs=1))
        ypool = ctx.enter_context(tc.tile_pool(name="ypool", bufs=ybufs))
        xspool = ctx.enter_context(tc.tile_pool(name="xspool", bufs=2))
        reppool = ctx.enter_context(tc.tile_pool(name="reppool", bufs=3))
        hpool = ctx.enter_context(tc.tile_pool(name="hpool", bufs=hbufs))
        umpool = ctx.enter_context(tc.tile_pool(name="umpool", bufs=3))
        opool = ctx.enter_context(tc.tile_pool(name="opool", bufs=2))
        psum = ctx.enter_context(tc.tile_pool(name="psum", bufs=3,
                                              space="PSUM"))
        upsum = ctx.enter_context(tc.tile_pool(name="upsum", bufs=2,
                                               space="PSUM"))
        dpool = ctx.enter_context(tc.tile_pool(name="dram", bufs=1,
                                               space="DRAM"))

        def _body(_it=None):
            # ---- constants ----
            WABC = const.tile([96, HID], ydt, tag="wabc")
            nc.sync.dma_start(WABC[:], wabc_ext[:])
            if y2:
                WABC2 = const.tile([32, HID], ydt, tag="wabc2")
                nc.sync.dma_start(WABC2[:], wabc_ext[64:96])
            W1 = const.tile([HID, 128], bf16, tag="w1x4")
            nc.sync.dma_start(W1[:], w1x4_ext[:])
            B0 = const.tile([HID, 1], f32, tag="b0")
            nc.sync.dma_start(B0[:], b0_ext[:])

            # ---- supertile: slab0 = x (w/ halos), slab1 = A, slab2 = B ----
            # x load and A/B prepass split into two row-chunks so the first
            # y-tiles can start while the second chunk computes
            SUP = big.tile([n_grp * C, 3, GR + 2, WP], ydt, tag="sup")
            HGR = GR // 2
            if sup_half and (n_grp * C // 2) % 32 == 0:
                # split load + prepass by group-halves (partition ranges) so
                # the next loop iteration's first-half load only WARs against
                # the first-half packs (drain hiding across reps)
                HP = n_grp * C // 2
                for hf in (0, HP):
                    nc.sync.dma_start(SUP[hf:hf + HP, 0, :, :],
                                      x_ext[hf:hf + HP])
                    if cut < 8:
                        sl = SUP[hf:hf + HP]
                        nc.vector.tensor_tensor(sl[:, 1, 1:1 + GR, :],
                                                sl[:, 0, 0:GR, :],
                                                sl[:, 0, 2:GR + 2, :],
                                                Alu.add)
                        nc.vector.scalar_tensor_tensor(
                            sl[:, 1, 1:1 + GR, :],
                            sl[:, 0, 1:GR + 1, :], 2.0,
                            sl[:, 1, 1:1 + GR, :], Alu.mult, Alu.add)
                        nc.vector.tensor_tensor(sl[:, 2, 1:1 + GR, :],
                                                sl[:, 0, 2:GR + 2, :],
                                                sl[:, 0, 0:GR, :],
                                                Alu.subtract)
            else:
                nc.sync.dma_start(SUP[:, 0, 0:HGR + 2, :],
                                  x_ext[:, 0:HGR + 2, :])
                nc.sync.dma_start(SUP[:, 0, HGR + 2:GR + 2, :],
                                  x_ext[:, HGR + 2:GR + 2, :])
                if cut < 8:
                    for r0, r1 in ((0, HGR), (HGR, GR)):
                        nc.vector.tensor_tensor(SUP[:, 1, 1 + r0:1 + r1, :],
                                                SUP[:, 0, r0:r1, :],
                                                SUP[:, 0, r0 + 2:r1 + 2, :],
                                                Alu.add)
                        nc.vector.scalar_tensor_tensor(
                            SUP[:, 1, 1 + r0:1 + r1, :],
                            SUP[:, 0, r0 + 1:r1 + 1, :], 2.0,
                            SUP[:, 1, 1 + r0:1 + r1, :], Alu.mult, Alu.add)
                        nc.vector.tensor_tensor(SUP[:, 2, 1 + r0:1 + r1, :],
                                                SUP[:, 0, r0 + 2:r1 + 2, :],
                                                SUP[:, 0, r0:r1, :],
                                                Alu.subtract)

            XSS = xss_box.get("t")

            if out2:
                # persistent ping-pong out tile; 2-y-tile out DMA granularity
                OF2 = big.tile([128, 2, UPY, W], bf16, tag="of2")

            evac_i = [0]
            pending_out = []        # deferred output DMAs

            def pending_emit(po):
                oyt, oOFY = po
                if out_split:
                    for k in range(4):
                        nc.sync.dma_start(out_ext[oyt, k],
                                          oOFY[32 * k:32 * k + 16])
                else:
                    nc.sync.dma_start(out_ext[oyt], oOFY[:, :, :])

            for yt in range(n_yt):
                g = (yt * YR) // GR
                lr = yt * YR - g * GR

                if (yt * UPY) % RCH == 0 and cut < 7:
                    REP = reppool.tile([128, RCH, W], mdt, tag="rep")
                    wi = (yt * UPY) // RCH
                    for k in range(4):
                        getattr(nc, rep_q).dma_start(
                            REP[32 * k:32 * k + 32, :, :],
                            mrep_ext[wi, k][None].broadcast_to([32, RCH, W]))
                if (yt * UPY) % XCH == 0 and cut < 7:
                    ch = (yt * UPY) // XCH
                    if xs_slim:
                        hb = (ch % 2) * XCH
                        for k in range(4):
                            getattr(nc, xs_q).dma_start(
                                XSS[32 * k:32 * k + XC, hb:hb + XCH, :],
                                xs_ext[ch, k])
                    else:
                        XS = xspool.tile([128, XCH, W], bf16, tag="xs")
                        for k in range(4):
                            getattr(nc, xs_q).dma_start(
                                XS[32 * k:32 * k + XC, :, :], xs_ext[ch, k])

                # ---- Y pack: flat [96, YR*514+2]; block flat offset 1-shift
                # Y pack: 3 DMAs; block pair interleaved per-channel so the
                # dst is a plain 32-partition slice (partition = 2c + slab,
                # matching the host-interleaved wabc rows)
                if cut >= 6:
                    continue
                NYP = 64 if y2 else 96
                Y = ypool.tile([NYP, YR * WP + 4], ydt, tag="y")
                rows = SUP[g * C:(g + 1) * C, :, lr + 1:lr + 1 + YR, :]
                if y2:
                    # dedup: AB pair packed once (block0); read at +0 and +2
                    # by two accumulating matmuls. xB block1 at skew 1.
                    blocks = (((1, 3, 1), 2, ypack_qs[0]),
                              ((0, 3, 2), 1, ypack_qs[1]))
                else:
                    blocks = (((1, 3, 1), 2, ypack_qs[0]),
                              ((0, 3, 2), 1, ypack_qs[1]),
                              ((1, 3, 1), 0, ypack_qs[2]))
                for bi, (slabs, off, eng) in enumerate(blocks):
                    s0, s1, st = slabs
                    src = rows[:, s0:s1:st, :, :]
                    dst = Y[32 * bi:32 * bi + 32, off:off + YR * WP]
                    getattr(nc, eng).dma_start(dst, src)

                for tl in range(UPY):
                    ut = yt * UPY + tl
                    hsb = []
                    if y2 and 'mm1' not in ablate:
                        # batch K=64 passes then K=32 passes: 2 stationary
                        # swaps per u-tile instead of 8
                        h2s = []
                        for _j in range(2):
                            h2t = psum.tile([HID, 2 * W], f32, tag="h2",
                                            name=f"h2_{ut}_{_j}")
                            h2s.append(h2t)
                        for j in range(2):
                            for r2 in range(2):
                                r = tl * UT + j * 2 + r2
                                nc.tensor.matmul(
                                    h2s[j][:, r2 * W:(r2 + 1) * W],
                                    WABC[0:64],
                                    Y[0:64, 2 + r * WP:2 + r * WP + W],
                                    start=True, stop=False)
                        for j in range(2):
                            for r2 in range(2):
                                r = tl * UT + j * 2 + r2
                                nc.tensor.matmul(
                                    h2s[j][:, r2 * W:(r2 + 1) * W],
                                    WABC2[:],
                                    Y[0:32, 4 + r * WP:4 + r * WP + W],
                                    start=False, stop=True)
                    else:
                        h2s = []
                    for j in range(2):
                        if y2 and 'mm1' not in ablate:
                            h2 = h2s[j]
                        else:
                            h2 = psum.tile([HID, 2 * W], f32, tag="h2")
                            for r2 in range(2):
                                r = tl * UT + j * 2 + r2  # local row in y-tile
                                if 'mm1' not in ablate:
                                    nc.tensor.matmul(
                                        h2[:, r2 * W:(r2 + 1) * W], WABC[:],
                                        Y[:, 2 + r * WP:2 + r * WP + W],
                                        start=True, stop=True)
                        hs = hpool.tile([HID, 2 * W], bf16, tag="h")
                        hsb.append(hs)
                        if 'evac' in ablate:
                            evac_i[0] += 1
                            continue
                        ph = (evac_ph,) if isinstance(evac_ph, int) else evac_ph
                        if evac_i[0] % evac_mod in ph:
                            nc.vector.tensor_scalar(hs[:], h2[:], B0[:], 0.0,
                                                    Alu.add, Alu.max)
                        else:
                            nc.scalar.activation(hs[:], h2[:], Act.Relu,
                                                 bias=B0[:])
                        evac_i[0] += 1
                    u_ps = upsum.tile([128, W], f32, tag="u")
                    for k in range(4):
                        if 'mm2' in ablate:
                            break
                        nc.tensor.matmul(u_ps[32 * k:32 * k + 32, :],
                                         W1[:, 32 * k:32 * k + 32],
                                         hsb[k // 2][:, (k % 2) * W:
                                                     (k % 2 + 1) * W],
                                         start=True, stop=True,
                                         tile_position=(0, 32 * k))
                    UM = umpool.tile([128, W], bf16, tag="um")
                    if 'um' not in ablate:
                        nc.vector.scalar_tensor_tensor(
                            UM[:], u_ps[:], 0.0, REP[:, ut % RCH, :],
                            Alu.bypass, Alu.mult)
                    if tl == 0:
                        if out2:
                            OFY = OF2[:, yt % 2]
                        else:
                            OFY = opool.tile([128, UPY, W], bf16, tag="ofy")
                    if 'of' not in ablate:
                        if xs_slim:
                            xcol = ((ut // XCH) % 2) * XCH + ut % XCH
                            xsrc = XSS[:, xcol, :]
                        else:
                            xsrc = XS[:, ut % XCH, :]
                        nc.gpsimd.tensor_tensor(OFY[:, tl, :], UM[:],
                                                xsrc, Alu.add)

                # defer out DMA by one y-tile so its sem wait doesn't stall
                # the SP sequencer ahead of the next y-tile's pack DMAs
                if 'out' in ablate:
                    continue
                if out2:
                    if yt % 2 == 1:
                        for k in range(4):
                            nc.sync.dma_start(out_ext[yt // 2, k],
                                              OF2[32 * k:32 * k + 16])
                    continue
                pending_out.append((yt, OFY))
                if len(pending_out) > 1:
                    pending_emit(pending_out.pop(0))
            for po in pending_out:
                pending_emit(po)

        # one-time (outside the reps loop): persistent xs tile + junk prime
        xss_box = {}
        if xs_slim:
            XSSp = big.tile([128, 2 * XCH, W], bf16, tag="xss")
            for k in range(4):
                for hb in (0, XCH):
                    nc.sync.dma_start(
                        XSSp[32 * k + XC:32 * k + 32, hb:hb + XCH, :],
                        xs_ext[0, 0][0:1].broadcast_to([32 - XC, XCH, W]))
            xss_box["t"] = XSSp

        if reps > 1:
            with tc.For_i(0, reps, 1):
                _body()
        else:
            _body()
    return nc


_CACHE = {}
CFG = dict(rep_q="gpsimd", out_split=True, xs_slim=True, xs_q="gpsimd",
           mask_fp8=True, evac_mod=5, evac_ph=(2,), xch=8, rch=8,
           y_fp8=False, y2=False, sup_half=False, out2=False, ybufs=3)


def _get_nc(**kw):
    kw = {**CFG, **kw}
    key = ("nc",) + tuple(sorted(kw.items()))
    if key not in _CACHE:
        nc = build_nc(**kw)
        nc.finalize()
        _CACHE[key] = nc
    return _CACHE[key]


def _to_bf16(a):
    import jax.numpy as jnp
    return np.asarray(jnp.asarray(a, dtype=jnp.bfloat16))


def _from_bf16(a):
    import jax.numpy as jnp
    return np.asarray(jnp.asarray(a), dtype=np.float32)


def _to_ydt(a):
    if CFG.get("y_fp8"):
        import ml_dtypes
        return np.asarray(a, np.float32).astype(ml_dtypes.float8_e4m3)
    return _to_bf16(a)


def _pack_weights(w0, b0, w1):
    w0 = np.asarray(w0, np.float32)
    w0x, w0gx, w0gy = w0[:, 0:C], w0[:, C:2 * C], w0[:, 2 * C:3 * C]
    def _ilv(a, b):                 # rows 2c = a[c], 2c+1 = b[c]
        return np.stack([a, b], axis=1).reshape(2 * C, HID)
    wabc = np.concatenate([
        _ilv(-w0gx.T, w0gy.T),      # [A|B interleaved] @ shift -1
        _ilv(w0x.T, 2.0 * w0gy.T),  # [x|B] @ shift 0
        _ilv(w0gx.T, w0gy.T),       # [A|B] @ shift +1
    ], axis=0)                      # [96, HID]
    w1x4 = np.zeros((HID, 128), np.float32)
    for k in range(4):
        w1x4[:, 32 * k:32 * k + C] = np.asarray(w1, np.float32).T
    b0 = np.ascontiguousarray(b0, np.float32).reshape(HID, 1)
    return _to_ydt(wabc), _to_bf16(w1x4), b0


def _mask_rep(rus, R, W, RCH=8):
    """[R, W] rand -> [n_rch, 4, RCH, W] compact mask.

    mrep[w, k, t, :] = (rus[RCH*UT*w + UT*t + k] > 0.5); the kernel
    broadcasts each [RCH, W] row-plane to 32 partitions via stride-0 DMA."""
    n_rch = (R // UT) // RCH
    m = (rus > 0.5).astype(np.float32)                    # [R, W]
    mr = m.reshape(n_rch, RCH, UT, W).transpose(0, 2, 1, 3)  # [w, k, t, W]
    mr = np.ascontiguousarray(mr)
    if CFG.get("mask_fp8"):
        import ml_dtypes
        return mr.astype(ml_dtypes.float8_e4m3)
    return _to_bf16(mr)


def _shard_inputs(x, w0, b0, w1, rand_u, R=256, W=512, GR=32, YR=16):
    B, _, H, Wf = x.shape
    half = H // 2
    n_grp = R // GR
    n_yt = R // YR
    n_ut = R // UT
    XCH = min(CFG.get("xch", 8), n_ut)
    n_xch = n_ut // XCH
    RCH = min(CFG.get("rch", 8), n_ut)
    wabc, w1x4, b0p = _pack_weights(w0, b0, w1)
    xp = np.pad(np.asarray(x, np.float32),
                ((0, 0), (0, 0), (1, 1), (1, 1)))
    in_maps = []
    for core in range(N_CORES):
        b, hh = divmod(core, 2)
        xsl = xp[b, :, hh * half:hh * half + half + 2, :]   # [C, R+2, WP]
        xsl = xsl.transpose(1, 0, 2)                        # [R+2, C, WP]
        xg = np.stack([xsl[GR * g:GR * g + GR + 2] for g in range(n_grp)])
        xpk = np.ascontiguousarray(
            xg.transpose(0, 2, 1, 3).reshape(n_grp * C, GR + 2, W + 2))
        # xs: [chunk, k, XC, XCH, W]; row = chunk*4*XCH + 4*t + k
        xc = xp[b, :, hh * half + 1:hh * half + 1 + half, 1:1 + W]  # [C,R,W]
        rows = xc.transpose(1, 0, 2).reshape(n_xch, XCH, 4, C, W)
        xs_r = np.ascontiguousarray(rows.transpose(0, 2, 3, 1, 4))
        if CFG.get("xs_slim"):
            xs = xs_r
        else:
            xs = np.zeros((n_xch, 4, 32, XCH, W), np.float32)
            xs[:, :, :C, :, :] = xs_r
        rus = rand_u[b, 0, hh * half:(hh + 1) * half, :].astype(np.float32)
        in_maps.append({
            "x": _to_ydt(xpk), "xs": _to_bf16(xs),
            "mrep": _mask_rep(rus, R, W, RCH),
            "wabc": wabc, "w1x4": w1x4, "b0": b0p})
    return in_maps


def _assemble(results, B, H, Wf, R=256, W=512, YR=16):
    out = np.empty((B, C, H, Wf), np.float32)
    half = H // 2
    n_yt = R // YR
    UPY = YR // UT
    for core, res in enumerate(results):
        b, hh = divmod(core, 2)
        if CFG.get("out2"):
            o = _from_bf16(res["out"])          # [n_p, 4, C, 2, UPY, W]
            o = o.transpose(0, 3, 4, 1, 2, 5).reshape(R, C, W)
        else:
            o = _from_bf16(res["out"])[:, :, :C]  # [n_yt, 4, C, UPY, W]
            o = o.transpose(0, 3, 1, 2, 4).reshape(R, C, W)  # row=16yt+4tl+k
        out[b, :, hh * half:(hh + 1) * half, :] = o.transpose(1, 0, 2)
    return out


def kernel(x, w0, b0, w1, rand_u, _trace=False):
    from concourse.bass_utils import run_bass_kernel_spmd
    nc = _get_nc()
    in_maps = _shard_inputs(x, w0, b0, w1, rand_u)
    res = run_bass_kernel_spmd(nc, in_maps, core_ids=list(range(N_CORES)))
    out = _assemble(res.results, x.shape[0], x.shape[2], x.shape[3])
    if _trace:
        return out, res
    return out


def _run_timed(nc, in_maps, iters):
    import time
    import jax
    from concourse import mybir
    from jax.sharding import Mesh, PartitionSpec
    from jax.experimental.shard_map import shard_map
    from concourse import bass2jax
    from concourse.bass2jax import _bass_exec_p

    bass2jax.install_neuronx_cc_hook()

    pname = nc.partition_id_tensor.name if nc.partition_id_tensor else None
    in_names, out_names, out_avals, zero_outs = [], [], [], []
    for alloc in nc.m.functions[0].allocations:
        if not isinstance(alloc, mybir.MemoryLocationSet):
            continue
        name = alloc.memorylocations[0].name
        if alloc.kind == "ExternalInput":
            if name != pname:
                in_names.append(name)
        elif alloc.kind == "ExternalOutput":
            out_names.append(name)
            shape = tuple(alloc.tensor_shape)
            np_dt = mybir.dt.np(alloc.dtype)
            out_avals.append(jax.core.ShapedArray(shape, np_dt))
            zero_outs.append(np.zeros(shape, np_dt))
    n_params = len(in_names)
    all_in = in_names + out_names
    if pname is not None:
        all_in = all_in + [pname]

    def _bodyfn(*args):
        operands = list(args)
        if pname is not None:
            operands.append(bass2jax.partition_id_tensor())
        outs = _bass_exec_p.bind(
            *operands, out_avals=tuple(out_avals), in_names=tuple(all_in),
            out_names=tuple(out_names), lowering_input_output_aliases=(),
            sim_require_finite=True, sim_require_nnan=True, nc=nc)
        return tuple(outs)

    devices = jax.devices()[:N_CORES]
    mesh = Mesh(np.asarray(devices), ("core",))
    specs = (PartitionSpec("core"),)
    fn = jax.jit(shard_map(_bodyfn, mesh=mesh,
                           in_specs=specs * (n_params + len(out_names)),
                           out_specs=specs * len(out_names), check_rep=False),
                 keep_unused=True)
    concat_in = [np.concatenate([np.asarray(in_maps[c][n])
                                 for c in range(N_CORES)], axis=0)
                 for n in in_names]
    concat_zeros = [np.zeros((N_CORES * z.shape[0], *z.shape[1:]), z.dtype)
                    for z in zero_outs]
    dev_in = [jax.device_put(a) for a in concat_in + concat_zeros]

    outs = fn(*dev_in)
    jax.block_until_ready(outs)
    best = float("inf")
    for _ in range(iters):
        t0 = time.perf_counter()
        outs = fn(*dev_in)
        jax.block_until_ready(outs)
        best = min(best, time.perf_counter() - t0)

    res = [{n: np.asarray(outs[i]).reshape(N_CORES, *out_avals[i].shape)[c]
            for i, n in enumerate(out_names)} for c in range(N_CORES)]
    return res, best


_REPS = 1025


def kernel_timed(x, w0, b0, w1, rand_u, iters=12):
    """Returns (out, est_exec_seconds): marginal per-iteration silicon time."""
    in_maps = _shard_inputs(x, w0, b0, w1, rand_u)
    nc1 = _get_nc()
    res, t1 = _run_timed(nc1, in_maps, iters)
    out = _assemble(res, x.shape[0], x.shape[2], x.shape[3])
    resR, tR = _run_timed(_get_nc(reps=_REPS), in_maps, iters)
    outR = _assemble(resR, x.shape[0], x.shape[2], x.shape[3])
    assert np.array_equal(out, outR), "reps variant output mismatch"
    est = (tR - t1) / (_REPS - 1)
    print(f"[timing] wall reps=1: {t1*1e6:.0f} us, reps={_REPS}: {tR*1e6:.0f} us"
          f" -> per-iter {est*1e6:.1f} us")
    return out, est


# ---------------- self-test (simulator, tiny geometry) ----------------
def _ref_numpy(x, w0, b0, w1, rand_u):
    sx = np.array([[-1, 0, 1], [-2, 0, 2], [-1, 0, 1]], np.float32)
    sy = sx.T
    Cc, H, Wf = x.shape
    xp = np.pad(x, ((0, 0), (1, 1), (1, 1)))
    gx = np.zeros_like(x)
    gy = np.zeros_like(x)
    for dy in range(3):
        for dx in range(3):
            gx += sx[dy, dx] * xp[:, dy:dy + H, dx:dx + Wf]
            gy += sy[dy, dx] * xp[:, dy:dy + H, dx:dx + Wf]
    y = np.concatenate([x, gx, gy], 0).reshape(3 * Cc, -1)
    h = np.maximum(w0 @ y + b0.reshape(-1, 1), 0)
    u = (w1 @ h).reshape(Cc, H, Wf)
    m = (rand_u > 0.5).astype(np.float32)
    return x + u * m


if __name__ == "__main__":
    from concourse.bass_interp import CoreSim
    R, W, GR, YR = 16, 32, 8, 8
    WP = W + 2
    n_grp = R // GR
    n_ut = R // UT
    XCH = min(CFG.get("xch", 8), n_ut)
    n_xch = n_ut // XCH
    RCH = min(CFG.get("rch", 8), n_ut)
    nc = build_nc(R=R, W=W, GR=GR, YR=YR, **CFG)
    nc.finalize()
    sim = CoreSim(nc, require_finite=False, require_nnan=False)
    rng = np.random.default_rng(0)
    xfull = rng.standard_normal((R + 2, C, WP)).astype(np.float32)
    xfull[0] = xfull[-1] = 0.0
    xfull[:, :, 0] = xfull[:, :, -1] = 0.0
    xg = np.stack([xfull[GR * g:GR * g + GR + 2] for g in range(n_grp)])
    x_packed = np.ascontiguousarray(
        xg.transpose(0, 2, 1, 3).reshape(n_grp * C, GR + 2, WP))
    xc = xfull[1:R + 1, :, 1:WP - 1]                   # [R, C, W]
    rows = xc.reshape(n_xch, XCH, 4, C, W)
    xs_r = np.ascontiguousarray(rows.transpose(0, 2, 3, 1, 4))
    if CFG.get("xs_slim"):
        xs = xs_r
    else:
        xs = np.zeros((n_xch, 4, 32, XCH, W), np.float32)
        xs[:, :, :C, :, :] = xs_r
    ru = rng.random((R, W)).astype(np.float32)
    w0 = (rng.standard_normal((HID, 3 * C)) * 0.1).astype(np.float32)
    b0 = (rng.standard_normal((HID, 1)) * 0.1).astype(np.float32)
    w1 = (rng.standard_normal((C, HID)) * 0.1).astype(np.float32)
    wabc, w1x4, b0p = _pack_weights(w0, b0.ravel(), w1)
    for n, v in [("x", _to_ydt(x_packed)), ("xs", _to_bf16(xs)),
                 ("mrep", _mask_rep(ru, R, W, RCH)), ("wabc", wabc),
                 ("w1x4", w1x4), ("b0", b0p)]:
        sim.tensor(n)[:] = v
    sim.simulate()
    o = _from_bf16(np.array(sim.tensor("out")))
    UPY = YR // UT
    if CFG.get("out2"):
        got = o.transpose(0, 3, 4, 1, 2, 5).reshape(R, C, W).transpose(1, 0, 2)
    else:
        got = o[:, :, :C].transpose(0, 3, 1, 2, 4).reshape(
            R, C, W).transpose(1, 0, 2)
    xin = xfull[1:R + 1, :, 1:WP - 1].transpose(1, 0, 2)
    exp = _ref_numpy(xin, w0, b0.ravel(), w1, ru)
    d = got - exp
    rel = np.linalg.norm(d) / np.linalg.norm(exp)
    print("L2 rel err:", rel, "absmax-scale:",
          np.abs(d).max() / np.abs(exp).max())
    assert rel < 2e-2, "FAIL"
    print("SIM PASS")



# revision 99
# speedup vs baseline: 1.4079x; 1.4079x over previous
"""Trainium2 Bass kernel for nn_CAModel (sobel-conv + 2-layer MLP + masked
residual).

Math per pixel: y = [x, sobel_x(x), sobel_y(x)] (48 ch); h = relu(w0 @ y + b0);
u = w1 @ h; out = x + u * (rand_u > 0.5).

Sharding: pure data-parallel over 8 cores: (batch b, H-half) -> core b*2 + half.
Each core computes a [16, 256, 512] slice of the output.

Design (v3): this environment is DMA-transfer bound (~120 GB/s effective,
all queues serialized), so everything is batched into few, large DMAs:
- mm1 as ONE K=96 matmul per row: Y96 = [A;B | x;B | A;B] where A = Sy x
  (vertical smooth), B = Dy x (vertical diff), computed by 3 DVE ops.
  Horizontal sobel shifts are baked into the pack DMAs via flat column
  offsets (Y stored flat [96, YR*514+2]; block pair at flat offset 1-shift).
  Block pairs are channel-interleaved (partition = 16*pair + 2c + slab) so
  each pack DMA has a plain partition-range destination.
- Residual x from a host-packed DRAM tensor in the stacked [32k+c] layout,
  loaded 16-partitions-per-block into a persistent double-wide tile whose
  junk partitions are primed once (v4: halves xs DMA bytes).
- Mask: host-thresholded compact fp8 [n_rch, 4, RCH, W] tensor, replicated
  on-chip with stride-0 partition-broadcast DMAs (v4: no device compare,
  no DRAM round-trip, half the mask bytes).
- Output: bf16 [yt, k, 16, tl, w]; only the 16 real channels of each
  32-block are DMA'd out (4 DMAs per y-tile); host converts to f32 and
  reassembles.
- Evac (PSUM->SBUF relu+bias) in FD=1024 ops split ACT/DVE (1/5 DVE);
  mask-multiply on DVE; residual add on GpSimd; out DMAs deferred one
  y-tile to avoid head-of-line blocking on the SP DMA queue.

HW attribution (ablations, per 326us iter): input-side DMAs ~92us,
Y-pack SBUF->SBUF ~189us (25.3MB at ~135GB/s effective), mm1+evac ~66us,
mm2/UM/OFY hidden, out DMAs ~47us. Effective DMA bandwidth is ~135-180
GB/s (not the 360 the cost model assumes), so bytes moved dominate.
Measured dead ends: fp8 Y-pack (gradients carry 96% of h variance ->
3.6% rel err, fails 2e-2), K=64+32 Y dedup (PE stationary-swap drains
cost more than the 8.4MB saved; PSUM zero-region rules forbid batched
accumulation), pre-replicated 8MB host mask (extra HBM reads), coarser
XS/REP chunks (hurt prefetch pipelining), sup_half drain-hiding (no
measurable gain), DVE partition-broadcast mask-on-h (stride-0 partition
APs illegal on compute engines), 2-y-tile out batching (neutral).
Run-to-run HW noise is +/-20us across processes.

Final config (CFG): out_split + xs_slim + fp8 compact host mask +
ybufs=3 (triple-buffered Y-pack). Best measured: 303.7us; typical
~304-325us vs 326-355us for the original baseline re-measured in the
same sessions.
"""
import numpy as np
from contextlib import ExitStack

import concourse.bass as bass
import concourse.bacc as bacc
import concourse.tile as tile
from concourse import mybir

bf16 = mybir.dt.bfloat16
f32 = mybir.dt.float32
Alu = mybir.AluOpType
Act = mybir.ActivationFunctionType

C = 16          # channels
HID = 128
N_CORES = 8
UT = 4          # rows per u-tile


def build_nc(R=256, W=512, GR=32, YR=16, reps=1, ablate=(), cut=0,
             rep_q="gpsimd", out_split=True, xs_slim=False, xs_q="gpsimd",
             mask_fp8=False, ypack_qs=("sync", "scalar", "gpsimd"),
             evac_mod=5, evac_ph=2, xch=8, rch=8, hbufs=4, y_fp8=False,
             y2=False, sup_half=False, out2=False, ybufs=2, choist=False):
    """Per-core graph. R out rows, W out cols, GR rows/group, YR rows/y-tile."""
    WP = W + 2
    n_grp = R // GR
    n_yt = R // YR
    n_ut = R // UT
    UPY = YR // UT                  # u-tiles per y-tile
    XCH = min(xch, n_ut)            # u-tiles per XS chunk
    n_xch = n_ut // XCH
    RCH = min(rch, n_ut)            # u-tiles per REP tile
    n_rch = n_ut // RCH
    SLAB = (GR + 2) * WP            # supertile slab elems per partition
    assert n_grp * C <= 128 and YR <= GR and GR % YR == 0 and YR % UT == 0

    nc = bacc.Bacc()
    XC = 16 if xs_slim else 32
    mdt = mybir.dt.float8e4 if mask_fp8 else bf16
    ydt = mybir.dt.float8e4 if y_fp8 else bf16
    x_ext = nc.declare_dram_parameter("x", (n_grp * C, GR + 2, WP), ydt,
                                      isOutput=False)
    xs_ext = nc.declare_dram_parameter("xs", (n_xch, 4, XC, XCH, W), bf16,
                                       isOutput=False)
    mrep_ext = nc.declare_dram_parameter("mrep", (n_rch, 4, RCH, W), mdt,
                                         isOutput=False)
    wabc_ext = nc.declare_dram_parameter("wabc", (96, HID), ydt,
                                         isOutput=False)
    w1x4_ext = nc.declare_dram_parameter("w1x4", (HID, 128), bf16,
                                         isOutput=False)
    b0_ext = nc.declare_dram_parameter("b0", (HID, 1), f32, isOutput=False)
    OC = 16 if out_split else 32
    if out2:
        assert out_split and n_yt % 2 == 0
        out_ext = nc.declare_dram_parameter(
            "out", (n_yt // 2, 4, 16, 2, UPY, W), bf16, isOutput=True)
    else:
        out_ext = nc.declare_dram_parameter(
            "out", (n_yt, 4, OC, UPY, W), bf16, isOutput=True)

    with tile.TileContext(nc) as tc, ExitStack() as ctx:
        const = ctx.enter_context(tc.tile_pool(name="const", bufs=1))
        big = ctx.enter_context(tc.tile_pool(name="big", bufs=1))
        ypool = ctx.enter_context(tc.tile_pool(name="ypool", bufs=ybufs))
        xspool = ctx.enter_context(tc.tile_pool(name="xspool", bufs=2))
        reppool = ctx.enter_context(tc.tile_pool(name="reppool", bufs=3))
        hpool = ctx.enter_context(tc.tile_pool(name="hpool", bufs=hbufs))
        umpool = ctx.enter_context(tc.tile_pool(name="umpool", bufs=3))
        opool = ctx.enter_context(tc.tile_pool(name="opool", bufs=2))
        psum = ctx.enter_context(tc.tile_pool(name="psum", bufs=3,
                                              space="PSUM"))
        upsum = ctx.enter_context(tc.tile_pool(name="upsum", bufs=2,
                                               space="PSUM"))
        dpool = ctx.enter_context(tc.tile_pool(name="dram", bufs=1,
                                               space="DRAM"))

        def _load_consts(box):
            WABCt = const.tile([96, HID], ydt, tag="wabc")
            nc.sync.dma_start(WABCt[:], wabc_ext[:])
            box["wabc"] = WABCt
            if y2:
                WABC2t = const.tile([32, HID], ydt, tag="wabc2")
                nc.sync.dma_start(WABC2t[:], wabc_ext[64:96])
                box["wabc2"] = WABC2t
            W1t = const.tile([HID, 128], bf16, tag="w1x4")
            nc.sync.dma_start(W1t[:], w1x4_ext[:])
            box["w1"] = W1t
            B0t = const.tile([HID, 1], f32, tag="b0")
            nc.sync.dma_start(B0t[:], b0_ext[:])
            box["b0"] = B0t

        def _body(_it=None):
            # ---- constants (hoisted outside the reps loop if choist) ----
            if not choist:
                _load_consts(cbox)
            WABC = cbox["wabc"]
            WABC2 = cbox.get("wabc2")
            W1 = cbox["w1"]
            B0 = cbox["b0"]

            # ---- supertile: slab0 = x (w/ halos), slab1 = A, slab2 = B ----
            # x load and A/B prepass split into two row-chunks so the first
            # y-tiles can start while the second chunk computes
            SUP = big.tile([n_grp * C, 3, GR + 2, WP], ydt, tag="sup")
            HGR = GR // 2
            if sup_half and (n_grp * C // 2) % 32 == 0:
                # split load + prepass by group-halves (partition ranges) so
                # the next loop iteration's first-half load only WARs against
                # the first-half packs (drain hiding across reps)
                HP = n_grp * C // 2
                for hf in (0, HP):
                    nc.sync.dma_start(SUP[hf:hf + HP, 0, :, :],
                                      x_ext[hf:hf + HP])
                    if cut < 8:
                        sl = SUP[hf:hf + HP]
                        nc.vector.tensor_tensor(sl[:, 1, 1:1 + GR, :],
                                                sl[:, 0, 0:GR, :],
                                                sl[:, 0, 2:GR + 2, :],
                                                Alu.add)
                        nc.vector.scalar_tensor_tensor(
                            sl[:, 1, 1:1 + GR, :],
                            sl[:, 0, 1:GR + 1, :], 2.0,
                            sl[:, 1, 1:1 + GR, :], Alu.mult, Alu.add)
                        nc.vector.tensor_tensor(sl[:, 2, 1:1 + GR, :],
                                                sl[:, 0, 2:GR + 2, :],
                                                sl[:, 0, 0:GR, :],
                                                Alu.subtract)
            else:
                nc.sync.dma_start(SUP[:, 0, 0:HGR + 2, :],
                                  x_ext[:, 0:HGR + 2, :])
                nc.sync.dma_start(SUP[:, 0, HGR + 2:GR + 2, :],
                                  x_ext[:, HGR + 2:GR + 2, :])
                if cut < 8:
                    for r0, r1 in ((0, HGR), (HGR, GR)):
                        nc.vector.tensor_tensor(SUP[:, 1, 1 + r0:1 + r1, :],
                                                SUP[:, 0, r0:r1, :],
                                                SUP[:, 0, r0 + 2:r1 + 2, :],
                                                Alu.add)
                        nc.vector.scalar_tensor_tensor(
                            SUP[:, 1, 1 + r0:1 + r1, :],
                            SUP[:, 0, r0 + 1:r1 + 1, :], 2.0,
                            SUP[:, 1, 1 + r0:1 + r1, :], Alu.mult, Alu.add)
                        nc.vector.tensor_tensor(SUP[:, 2, 1 + r0:1 + r1, :],
                                                SUP[:, 0, r0 + 2:r1 + 2, :],
                                                SUP[:, 0, r0:r1, :],
                                                Alu.subtract)

            XSS = xss_box.get("t")

            if out2:
                # persistent ping-pong out tile; 2-y-tile out DMA granularity
                OF2 = big.tile([128, 2, UPY, W], bf16, tag="of2")

            evac_i = [0]
            pending_out = []        # deferred output DMAs

            def pending_emit(po):
                oyt, oOFY = po
                if out_split:
                    for k in range(4):
                        nc.sync.dma_start(out_ext[oyt, k],
                                          oOFY[32 * k:32 * k + 16])
                else:
                    nc.sync.dma_start(out_ext[oyt], oOFY[:, :, :])

            for yt in range(n_yt):
                g = (yt * YR) // GR
                lr = yt * YR - g * GR

                if (yt * UPY) % RCH == 0 and cut < 7:
                    REP = reppool.tile([128, RCH, W], mdt, tag="rep")
                    wi = (yt * UPY) // RCH
                    for k in range(4):
                        getattr(nc, rep_q).dma_start(
                            REP[32 * k:32 * k + 32, :, :],
                            mrep_ext[wi, k][None].broadcast_to([32, RCH, W]))
                if (yt * UPY) % XCH == 0 and cut < 7:
                    ch = (yt * UPY) // XCH
                    if xs_slim:
                        hb = (ch % 2) * XCH
                        for k in range(4):
                            getattr(nc, xs_q).dma_start(
                                XSS[32 * k:32 * k + XC, hb:hb + XCH, :],
                                xs_ext[ch, k])
                    else:
                        XS = xspool.tile([128, XCH, W], bf16, tag="xs")
                        for k in range(4):
                            getattr(nc, xs_q).dma_start(
                                XS[32 * k:32 * k + XC, :, :], xs_ext[ch, k])

                # ---- Y pack: flat [96, YR*514+2]; block flat offset 1-shift
                # Y pack: 3 DMAs; block pair interleaved per-channel so the
                # dst is a plain 32-partition slice (partition = 2c + slab,
                # matching the host-interleaved wabc rows)
                if cut >= 6:
                    continue
                NYP = 64 if y2 else 96
                Y = ypool.tile([NYP, YR * WP + 4], ydt, tag="y")
                rows = SUP[g * C:(g + 1) * C, :, lr + 1:lr + 1 + YR, :]
                if y2:
                    # dedup: AB pair packed once (block0); read at +0 and +2
                    # by two accumulating matmuls. xB block1 at skew 1.
                    blocks = (((1, 3, 1), 2, ypack_qs[0]),
                              ((0, 3, 2), 1, ypack_qs[1]))
                else:
                    blocks = (((1, 3, 1), 2, ypack_qs[0]),
                              ((0, 3, 2), 1, ypack_qs[1]),
                              ((1, 3, 1), 0, ypack_qs[2]))
                for bi, (slabs, off, eng) in enumerate(blocks):
                    s0, s1, st = slabs
                    src = rows[:, s0:s1:st, :, :]
                    dst = Y[32 * bi:32 * bi + 32, off:off + YR * WP]
                    getattr(nc, eng).dma_start(dst, src)

                for tl in range(UPY):
                    ut = yt * UPY + tl
                    hsb = []
                    if y2 and 'mm1' not in ablate:
                        # batch K=64 passes then K=32 passes: 2 stationary
                        # swaps per u-tile instead of 8
                        h2s = []
                        for _j in range(2):
                            h2t = psum.tile([HID, 2 * W], f32, tag="h2",
                                            name=f"h2_{ut}_{_j}")
                            h2s.append(h2t)
                        for j in range(2):
                            for r2 in range(2):
                                r = tl * UT + j * 2 + r2
                                nc.tensor.matmul(
                                    h2s[j][:, r2 * W:(r2 + 1) * W],
                                    WABC[0:64],
                                    Y[0:64, 2 + r * WP:2 + r * WP + W],
                                    start=True, stop=False)
                        for j in range(2):
                            for r2 in range(2):
                                r = tl * UT + j * 2 + r2
                                nc.tensor.matmul(
                                    h2s[j][:, r2 * W:(r2 + 1) * W],
                                    WABC2[:],
                                    Y[0:32, 4 + r * WP:4 + r * WP + W],
                                    start=False, stop=True)
                    else:
                        h2s = []
                    for j in range(2):
                        if y2 and 'mm1' not in ablate:
                            h2 = h2s[j]
                        else:
                            h2 = psum.tile([HID, 2 * W], f32, tag="h2")
                            for r2 in range(2):
                                r = tl * UT + j * 2 + r2  # local row in y-tile
                                if 'mm1' not in ablate:
                                    nc.tensor.matmul(
                                        h2[:, r2 * W:(r2 + 1) * W], WABC[:],
                                        Y[:, 2 + r * WP:2 + r * WP + W],
                                        start=True, stop=True)
                        hs = hpool.tile([HID, 2 * W], bf16, tag="h")
                        hsb.append(hs)
                        if 'evac' in ablate:
                            evac_i[0] += 1
                            continue
                        ph = (evac_ph,) if isinstance(evac_ph, int) else evac_ph
                        if evac_i[0] % evac_mod in ph:
                            nc.vector.tensor_scalar(hs[:], h2[:], B0[:], 0.0,
                                                    Alu.add, Alu.max)
                        else:
                            nc.scalar.activation(hs[:], h2[:], Act.Relu,
                                                 bias=B0[:])
                        evac_i[0] += 1
                    u_ps = upsum.tile([128, W], f32, tag="u")
                    for k in range(4):
                        if 'mm2' in ablate:
                            break
                        nc.tensor.matmul(u_ps[32 * k:32 * k + 32, :],
                                         W1[:, 32 * k:32 * k + 32],
                                         hsb[k // 2][:, (k % 2) * W:
                                                     (k % 2 + 1) * W],
                                         start=True, stop=True,
                                         tile_position=(0, 32 * k))
                    UM = umpool.tile([128, W], bf16, tag="um")
                    if 'um' not in ablate:
                        nc.vector.scalar_tensor_tensor(
                            UM[:], u_ps[:], 0.0, REP[:, ut % RCH, :],
                            Alu.bypass, Alu.mult)
                    if tl == 0:
                        if out2:
                            OFY = OF2[:, yt % 2]
                        else:
                            OFY = opool.tile([128, UPY, W], bf16, tag="ofy")
                    if 'of' not in ablate:
                        if xs_slim:
                            xcol = ((ut // XCH) % 2) * XCH + ut % XCH
                            xsrc = XSS[:, xcol, :]
                        else:
                            xsrc = XS[:, ut % XCH, :]
                        nc.gpsimd.tensor_tensor(OFY[:, tl, :], UM[:],
                                                xsrc, Alu.add)

                # defer out DMA by one y-tile so its sem wait doesn't stall
                # the SP sequencer ahead of the next y-tile's pack DMAs
                if 'out' in ablate:
                    continue
                if out2:
                    if yt % 2 == 1:
                        for k in range(4):
                            nc.sync.dma_start(out_ext[yt // 2, k],
                                              OF2[32 * k:32 * k + 16])
                    continue
                pending_out.append((yt, OFY))
                if len(pending_out) > 1:
                    pending_emit(pending_out.pop(0))
            for po in pending_out:
                pending_emit(po)

        # one-time (outside the reps loop): consts (if choist) + xs prime
        cbox = {}
        if choist:
            _load_consts(cbox)
        xss_box = {}
        if xs_slim:
            XSSp = big.tile([128, 2 * XCH, W], bf16, tag="xss")
            for k in range(4):
                for hb in (0, XCH):
                    nc.sync.dma_start(
                        XSSp[32 * k + XC:32 * k + 32, hb:hb + XCH, :],
                        xs_ext[0, 0][0:1].broadcast_to([32 - XC, XCH, W]))
            xss_box["t"] = XSSp

        if reps > 1:
            with tc.For_i(0, reps, 1):
                _body()
        else:
            _body()
    return nc


_CACHE = {}
CFG = dict(rep_q="gpsimd", out_split=True, xs_slim=True, xs_q="gpsimd",
           mask_fp8=True, evac_mod=5, evac_ph=(2,), xch=8, rch=8,
           y_fp8=False, y2=False, sup_half=False, out2=False, ybufs=3,
           choist=False)


def _get_nc(**kw):
    kw = {**CFG, **kw}
    key = ("nc",) + tuple(sorted(kw.items()))
    if key not in _CACHE:
        nc = build_nc(**kw)
        nc.finalize()
        _CACHE[key] = nc
    return _CACHE[key]


def _to_bf16(a):
    import jax.numpy as jnp
    return np.asarray(jnp.asarray(a, dtype=jnp.bfloat16))


def _from_bf16(a):
    import jax.numpy as jnp
    return np.asarray(jnp.asarray(a), dtype=np.float32)


def _to_ydt(a):
    if CFG.get("y_fp8"):
        import ml_dtypes
        return np.asarray(a, np.float32).astype(ml_dtypes.float8_e4m3)
    return _to_bf16(a)


def _pack_weights(w0, b0, w1):
    w0 = np.asarray(w0, np.float32)
    w0x, w0gx, w0gy = w0[:, 0:C], w0[:, C:2 * C], w0[:, 2 * C:3 * C]
    def _ilv(a, b):                 # rows 2c = a[c], 2c+1 = b[c]
        return np.stack([a, b], axis=1).reshape(2 * C, HID)
    wabc = np.concatenate([
        _ilv(-w0gx.T, w0gy.T),      # [A|B interleaved] @ shift -1
        _ilv(w0x.T, 2.0 * w0gy.T),  # [x|B] @ shift 0
        _ilv(w0gx.T, w0gy.T),       # [A|B] @ shift +1
    ], axis=0)                      # [96, HID]
    w1x4 = np.zeros((HID, 128), np.float32)
    for k in range(4):
        w1x4[:, 32 * k:32 * k + C] = np.asarray(w1, np.float32).T
    b0 = np.ascontiguousarray(b0, np.float32).reshape(HID, 1)
    return _to_ydt(wabc), _to_bf16(w1x4), b0


def _mask_rep(rus, R, W, RCH=8):
    """[R, W] rand -> [n_rch, 4, RCH, W] compact mask.

    mrep[w, k, t, :] = (rus[RCH*UT*w + UT*t + k] > 0.5); the kernel
    broadcasts each [RCH, W] row-plane to 32 partitions via stride-0 DMA."""
    n_rch = (R // UT) // RCH
    m = (rus > 0.5).astype(np.float32)                    # [R, W]
    mr = m.reshape(n_rch, RCH, UT, W).transpose(0, 2, 1, 3)  # [w, k, t, W]
    mr = np.ascontiguousarray(mr)
    if CFG.get("mask_fp8"):
        import ml_dtypes
        return mr.astype(ml_dtypes.float8_e4m3)
    return _to_bf16(mr)


def _shard_inputs(x, w0, b0, w1, rand_u, R=256, W=512, GR=32, YR=16):
    B, _, H, Wf = x.shape
    half = H // 2
    n_grp = R // GR
    n_yt = R // YR
    n_ut = R // UT
    XCH = min(CFG.get("xch", 8), n_ut)
    n_xch = n_ut // XCH
    RCH = min(CFG.get("rch", 8), n_ut)
    wabc, w1x4, b0p = _pack_weights(w0, b0, w1)
    xp = np.pad(np.asarray(x, np.float32),
                ((0, 0), (0, 0), (1, 1), (1, 1)))
    in_maps = []
    for core in range(N_CORES):
        b, hh = divmod(core, 2)
        xsl = xp[b, :, hh * half:hh * half + half + 2, :]   # [C, R+2, WP]
        xsl = xsl.transpose(1, 0, 2)                        # [R+2, C, WP]
        xg = np.stack([xsl[GR * g:GR * g + GR + 2] for g in range(n_grp)])
        xpk = np.ascontiguousarray(
            xg.transpose(0, 2, 1, 3).reshape(n_grp * C, GR + 2, W + 2))
        # xs: [chunk, k, XC, XCH, W]; row = chunk*4*XCH + 4*t + k
        xc = xp[b, :, hh * half + 1:hh * half + 1 + half, 1:1 + W]  # [C,R,W]
        rows = xc.transpose(1, 0, 2).reshape(n_xch, XCH, 4, C, W)
        xs_r = np.ascontiguousarray(rows.transpose(0, 2, 3, 1, 4))
        if CFG.get("xs_slim"):
            xs = xs_r
        else:
            xs = np.zeros((n_xch, 4, 32, XCH, W), np.float32)
            xs[:, :, :C, :, :] = xs_r
        rus = rand_u[b, 0, hh * half:(hh + 1) * half, :].astype(np.float32)
        in_maps.append({
            "x": _to_ydt(xpk), "xs": _to_bf16(xs),
            "mrep": _mask_rep(rus, R, W, RCH),
            "wabc": wabc, "w1x4": w1x4, "b0": b0p})
    return in_maps


def _assemble(results, B, H, Wf, R=256, W=512, YR=16):
    out = np.empty((B, C, H, Wf), np.float32)
    half = H // 2
    n_yt = R // YR
    UPY = YR // UT
    for core, res in enumerate(results):
        b, hh = divmod(core, 2)
        if CFG.get("out2"):
            o = _from_bf16(res["out"])          # [n_p, 4, C, 2, UPY, W]
            o = o.transpose(0, 3, 4, 1, 2, 5).reshape(R, C, W)
        else:
            o = _from_bf16(res["out"])[:, :, :C]  # [n_yt, 4, C, UPY, W]
            o = o.transpose(0, 3, 1, 2, 4).reshape(R, C, W)  # row=16yt+4tl+k
        out[b, :, hh * half:(hh + 1) * half, :] = o.transpose(1, 0, 2)
    return out


def kernel(x, w0, b0, w1, rand_u, _trace=False):
    from concourse.bass_utils import run_bass_kernel_spmd
    nc = _get_nc()
    in_maps = _shard_inputs(x, w0, b0, w1, rand_u)
    res = run_bass_kernel_spmd(nc, in_maps, core_ids=list(range(N_CORES)))
    out = _assemble(res.results, x.shape[0], x.shape[2], x.shape[3])
    if _trace:
        return out, res
    return out


def _run_timed(nc, in_maps, iters):
    import time
    import jax
    from concourse import mybir
    from jax.sharding import Mesh, PartitionSpec
    from jax.experimental.shard_map import shard_map
    from concourse import bass2jax
    from concourse.bass2jax import _bass_exec_p

    bass2jax.install_neuronx_cc_hook()

    pname = nc.partition_id_tensor.name if nc.partition_id_tensor else None
    in_names, out_names, out_avals, zero_outs = [], [], [], []
    for alloc in nc.m.functions[0].allocations:
        if not isinstance(alloc, mybir.MemoryLocationSet):
            continue
        name = alloc.memorylocations[0].name
        if alloc.kind == "ExternalInput":
            if name != pname:
                in_names.append(name)
        elif alloc.kind == "ExternalOutput":
            out_names.append(name)
            shape = tuple(alloc.tensor_shape)
            np_dt = mybir.dt.np(alloc.dtype)
            out_avals.append(jax.core.ShapedArray(shape, np_dt))
            zero_outs.append(np.zeros(shape, np_dt))
    n_params = len(in_names)
    all_in = in_names + out_names
    if pname is not None:
        all_in = all_in + [pname]

    def _bodyfn(*args):
        operands = list(args)
        if pname is not None:
            operands.append(bass2jax.partition_id_tensor())
        outs = _bass_exec_p.bind(
            *operands, out_avals=tuple(out_avals), in_names=tuple(all_in),
            out_names=tuple(out_names), lowering_input_output_aliases=(),
            sim_require_finite=True, sim_require_nnan=True, nc=nc)
        return tuple(outs)

    devices = jax.devices()[:N_CORES]
    mesh = Mesh(np.asarray(devices), ("core",))
    specs = (PartitionSpec("core"),)
    fn = jax.jit(shard_map(_bodyfn, mesh=mesh,
                           in_specs=specs * (n_params + len(out_names)),
                           out_specs=specs * len(out_names), check_rep=False),
                 keep_unused=True)
    concat_in = [np.concatenate([np.asarray(in_maps[c][n])
                                 for c in range(N_CORES)], axis=0)
                 for n in in_names]
    concat_zeros = [np.zeros((N_CORES * z.shape[0], *z.shape[1:]), z.dtype)
                    for z in zero_outs]
    dev_in = [jax.device_put(a) for a in concat_in + concat_zeros]

    outs = fn(*dev_in)
    jax.block_until_ready(outs)
    best = float("inf")
    for _ in range(iters):
        t0 = time.perf_counter()
        outs = fn(*dev_in)
        jax.block_until_ready(outs)
        best = min(best, time.perf_counter() - t0)

    res = [{n: np.asarray(outs[i]).reshape(N_CORES, *out_avals[i].shape)[c]
            for i, n in enumerate(out_names)} for c in range(N_CORES)]
    return res, best


_REPS = 1025


def kernel_timed(x, w0, b0, w1, rand_u, iters=12):
    """Returns (out, est_exec_seconds): marginal per-iteration silicon time."""
    in_maps = _shard_inputs(x, w0, b0, w1, rand_u)
    nc1 = _get_nc()
    res, t1 = _run_timed(nc1, in_maps, iters)
    out = _assemble(res, x.shape[0], x.shape[2], x.shape[3])
    resR, tR = _run_timed(_get_nc(reps=_REPS), in_maps, iters)
    outR = _assemble(resR, x.shape[0], x.shape[2], x.shape[3])
    assert np.array_equal(out, outR), "reps variant output mismatch"
    est = (tR - t1) / (_REPS - 1)
    print(f"[timing] wall reps=1: {t1*1e6:.0f} us, reps={_REPS}: {tR*1e6:.0f} us"
          f" -> per-iter {est*1e6:.1f} us")
    return out, est


# ---------------- self-test (simulator, tiny geometry) ----------------
def _ref_numpy(x, w0, b0, w1, rand_u):
    sx = np.array([[-1, 0, 1], [-2, 0, 2], [-1, 0, 1]], np.float32)
    sy = sx.T
    Cc, H, Wf = x.shape
    xp = np.pad(x, ((0, 0), (1, 1), (1, 1)))
    gx = np.zeros_like(x)
    gy = np.zeros_like(x)
    for dy in range(3):
        for dx in range(3):
            gx += sx[dy, dx] * xp[:, dy:dy + H, dx:dx + Wf]
            gy += sy[dy, dx] * xp[:, dy:dy + H, dx:dx + Wf]
    y = np.concatenate([x, gx, gy], 0).reshape(3 * Cc, -1)
    h = np.maximum(w0 @ y + b0.reshape(-1, 1), 0)
    u = (w1 @ h).reshape(Cc, H, Wf)
    m = (rand_u > 0.5).astype(np.float32)
    return x + u * m


if __name__ == "__main__":
    from concourse.bass_interp import CoreSim
    R, W, GR, YR = 16, 32, 8, 8
    WP = W + 2
    n_grp = R // GR
    n_ut = R // UT
    XCH = min(CFG.get("xch", 8), n_ut)
    n_xch = n_ut // XCH
    RCH = min(CFG.get("rch", 8), n_ut)
    nc = build_nc(R=R, W=W, GR=GR, YR=YR, **CFG)
    nc.finalize()
    sim = CoreSim(nc, require_finite=False, require_nnan=False)
    rng = np.random.default_rng(0)
    xfull = rng.standard_normal((R + 2, C, WP)).astype(np.float32)
    xfull[0] = xfull[-1] = 0.0
    xfull[:, :, 0] = xfull[:, :, -1] = 0.0
    xg = np.stack([xfull[GR * g:GR * g + GR + 2] for g in range(n_grp)])
    x_packed = np.ascontiguousarray(
        xg.transpose(0, 2, 1, 3).reshape(n_grp * C, GR + 2, WP))
    xc = xfull[1:R + 1, :, 1:WP - 1]                   # [R, C, W]
    rows = xc.reshape(n_xch, XCH, 4, C, W)
    xs_r = np.ascontiguousarray(rows.transpose(0, 2, 3, 1, 4))
    if CFG.get("xs_slim"):
        xs = xs_r
    else:
        xs = np.zeros((n_xch, 4, 32, XCH, W), np.float32)
        xs[:, :, :C, :, :] = xs_r
    ru = rng.random((R, W)).astype(np.float32)
    w0 = (rng.standard_normal((HID, 3 * C)) * 0.1).astype(np.float32)
    b0 = (rng.standard_normal((HID, 1)) * 0.1).astype(np.float32)
    w1 = (rng.standard_normal((C, HID)) * 0.1).astype(np.float32)
    wabc, w1x4, b0p = _pack_weights(w0, b0.ravel(), w1)
    for n, v in [("x", _to_ydt(x_packed)), ("xs", _to_bf16(xs)),
                 ("mrep", _mask_rep(ru, R, W, RCH)), ("wabc", wabc),
                 ("w1x4", w1x4), ("b0", b0p)]:
        sim.tensor(n)[:] = v
    sim.simulate()
    o = _from_bf16(np.array(sim.tensor("out")))
    UPY = YR // UT
    if CFG.get("out2"):
        got = o.transpose(0, 3, 4, 1, 2, 5).reshape(R, C, W).transpose(1, 0, 2)
    else:
        got = o[:, :, :C].transpose(0, 3, 1, 2, 4).reshape(
            R, C, W).transpose(1, 0, 2)
    xin = xfull[1:R + 1, :, 1:WP - 1].transpose(1, 0, 2)
    exp = _ref_numpy(xin, w0, b0.ravel(), w1, ru)
    d = got - exp
    rel = np.linalg.norm(d) / np.linalg.norm(exp)
    print("L2 rel err:", rel, "absmax-scale:",
          np.abs(d).max() / np.abs(exp).max())
    assert rel < 2e-2, "FAIL"
    print("SIM PASS")

